# revision 14
# baseline (speedup 1.0000x reference)
"""Trainium2 Bass kernel for nn_FIoUCriterion (pairwise-overlap IoU-style loss).

Strategy (8 NeuronCores, data-parallel over batch), variant "fwl":
  - Host: y = max(x,-1)+1 (= 2*m), cast fp8 e3m4 (rel-err on final loss ~8e-5),
    pre-transpose per core to pixel-major layout [128, pair(2), ktile(128), 129]
    where each ktile block is [b_even nodes (64) | b_odd nodes (64) | ones].
  - Device (per core): per 128-pixel ktile and per 2-batch pair, ONE matmul
    with M=128 stacked weights (both batches) and N=129 streamed columns:
      out[128, 129] += block[:, 0:128].T @ block[:, 0:129]
    The 128-column weight enables Fast Weight Load (4 fp8/cycle), so the
    per-matmul cost is stream-bound (~N cycles) instead of load-bound --
    the old two-M=64-matmul structure paid two non-FWL 64-col loads per
    ktile (load-bound, ~2x slower).  Diagonal 64x64 blocks of the PSUM are
    the per-batch grams, col 128 is the mass sums s for both batches;
    off-diagonal blocks are unused cross-batch values (computed in the
    array's parallel rows -- zero extra cycles).
  - Epilogue per pair: C = gram/s_row per half, PE transposes per 64-block,
    cr_contrib = max(C, C^T) (valid since gram>=0), fold lower block via
    PE transpose (symmetric), AllGather + local-sum of the (64,64) partials
    across 8 cores, then loss = sum(|beta - cr_sum/64| * wgt2) with
    wgt2 = (wgt + wgt^T) / (2*sum(wgt)).
  - Scale bookkeeping: y = 2m  => gram_psum = 4*gram, s_psum = 2*s,
    C = 2*gram/s; sum over 32 batches then *1/64 gives mean cr.
  - DMA: plain contiguous loads, pair-major, alternated across the two HWDGE
    queues (SP + Activation).
"""

import numpy as np
import ml_dtypes

N_CORES = 8
B, N, H, W = 32, 64, 128, 128
HW = H * W
B_LOC = B // N_CORES          # 4 batches per core
N_PAIRS = B_LOC // 2          # 2 stacked pairs per core
N_K = HW // 128               # 128 ktiles of 128 pixels
CPK = 129                     # cols per ktile block (64 + 64 + ones)
N_SEPARATE = 7
N_FLEXIBLE = 2

_cached = {}


def _build_bass(with_collective: bool = True, bench_loop: int | None = None,
                phase: str = "full",
                calls_by_pair=None, dual_ring: bool = False,
                pe_order: str = "alt"):
    import contextlib
    import concourse.bacc as bacc
    import concourse.mybir as mybir
    import concourse.tile as tile

    f32 = mybir.dt.float32
    f8 = mybir.dt.float8e3
    Alu = mybir.AluOpType

    nc = bacc.Bacc("TRN2", target_bir_lowering=False, debug=False, num_devices=N_CORES)
    xt = nc.dram_tensor("xt", [128, N_PAIRS * N_K * CPK], f8, kind="ExternalInput")
    beta_d = nc.dram_tensor("beta", [N, N], f32, kind="ExternalInput")
    wgt2_d = nc.dram_tensor("wgt2", [N, N], f32, kind="ExternalInput")
    loss_d = nc.dram_tensor("loss", [1, 1], f32, kind="ExternalOutput")

    def emit(tc, const, stream, ep, gpsum, tpsum, dram):
        # --- constants ---
        ones_f32 = const.tile([N, 1], f32)
        nc.vector.memset(ones_f32[:], 1.0)
        ident = const.tile([128, 128], f32)
        from concourse import masks as masks_lib
        masks_lib.make_identity(nc, ident[:])
        beta_t = const.tile([N, N], f32)
        nc.sync.dma_start(beta_t[:], beta_d[:])
        wgt2_t = const.tile([N, N], f32)
        nc.sync.dma_start(wgt2_t[:], wgt2_d[:])

        # --- PSUM accumulators per pair: [128, 129] = [gram blocks | s] ---
        g_acc = [gpsum.tile([128, CPK], f32, name=f"g_acc{p}",
                            padded_shape=[128, 512]) for p in range(N_PAIRS)]

        bench_cm = (tc.For_i(0, bench_loop, 1, hint_engines=(mybir.EngineType.PE,))
                    if bench_loop else contextlib.nullcontext())
        bench_cm.__enter__()

        f0 = ep.tile([N, N], f32)        # pair0's folded (64,64) partial
        a1 = ep.tile([N, N], f32)
        crl = ep.tile([N, N], f32)

        def k_matmul(p, t, k, first, last):
            block = t[:, k, :]           # [128, 129]
            nc.tensor.matmul(g_acc[p][:, 0:CPK],
                             lhsT=block[:, 0:128], rhs=block,
                             start=first, stop=last)

        def pair_epilogue(p):
            g = g_acc[p]
            # C[i,:] = gram_block(i)/s_i, stacked [128, 64]
            r = ep.tile([128, 1], f32, name=f"r{p}")
            nc.vector.reciprocal(r[:], g[:, 128:129])
            C = ep.tile([128, 64], f32, name=f"C{p}")
            nc.vector.tensor_scalar_mul(C[0:64, :], g[0:64, 0:64], r[0:64])
            nc.vector.tensor_scalar_mul(C[64:128, :], g[64:128, 64:128], r[64:128])
            # one full-width transpose: CT2[:, 0:64] = C_top^T, [:, 64:128] =
            # C_bot^T -- all on partitions 0:64 (transpose out must be at
            # PSUM partition 0)
            CT2 = tpsum.tile([64, 128], f32, name=f"CT{p}", tag="CT", bufs=1,
                             padded_shape=[64, 512])
            nc.tensor.transpose(CT2[:], C[:], ident[:])
            mxt = ep.tile([N, N], f32, name=f"mxt{p}")
            nc.vector.tensor_max(mxt[:], C[0:64, :], CT2[:, 0:64])
            # bottom block: copy C_bot^T to SBUF, re-transpose for the other
            # orientation, then max -- already folded to partitions 0:64
            # (copy on Activation so it overlaps the DVE max above)
            cpB = ep.tile([N, N], f32, name=f"cpB{p}")
            nc.scalar.copy(cpB[:], CT2[:, 64:128])
            TB = tpsum.tile([N, N], f32, name=f"TB{p}", tag="TB", bufs=1,
                            padded_shape=[64, 512])
            nc.tensor.transpose(TB[:], cpB[:], ident[0:64, 0:64])
            mxb = ep.tile([N, N], f32, name=f"mxb{p}")
            nc.vector.tensor_max(mxb[:], cpB[:], TB[:])
            if p == 0:
                # runs while pair1 is still streaming -- free
                nc.vector.tensor_add(f0[:], mxt[:], mxb[:])
            else:
                nc.vector.tensor_add(a1[:], f0[:], mxt[:])
                nc.vector.tensor_add(crl[:], a1[:], mxb[:])

        if phase == "pe":
            # pure PE-rate probe: one resident tile, full matmul count
            Xc = 16
            t = stream.tile([128, Xc, CPK], f8, name="tpe", tag="tpe", bufs=1)
            nc.sync.dma_start(t[:], xt[:, 0:Xc * CPK])
            if pe_order == "alt":
                for rep in range(N_K // Xc):
                    for k in range(Xc):
                        first = (rep == 0 and k == 0)
                        last = (rep == N_K // Xc - 1 and k == Xc - 1)
                        for p in range(N_PAIRS):
                            k_matmul(p, t, k, first, last)
            else:  # pair-major: replicate the stream phase's matmul order
                for p in range(N_PAIRS):
                    for rep in range(N_K // Xc):
                        for k in range(Xc):
                            first = (rep == 0 and k == 0)
                            last = (rep == N_K // Xc - 1 and k == Xc - 1)
                            k_matmul(p, t, k, first, last)
            lout0 = ep.tile([1, 1], f32)
            nc.vector.memset(lout0[:], 0.0)
            nc.sync.dma_start(loss_d[:], lout0[:])
            bench_cm.__exit__(None, None, None)
            return

        # --- streaming: plain contiguous DMA, pair-major; pair0's epilogue
        # overlaps pair1's matmul stream.  Calls alternate between the two
        # HWDGE queues (SP / Activation) when dual_ring. ---
        # Call shaping: ramped sizes; very large calls regress DMA throughput
        # (measured: a single 94-ktile call runs ~25% slower than 32-40 ktile
        # calls), so stay in the 12-40 ktile range after the pipe-fill call.
        CALLS_BY_PAIR = calls_by_pair or [
            [4, 12, 32, 40, 40],
            [40, 40, 32, 16],
        ]
        qi = 0
        for p in range(N_PAIRS if phase != "noop" else 0):
            CALLS = CALLS_BY_PAIR[p]
            assert sum(CALLS) == N_K
            c0 = 0
            for ci, Xc in enumerate(CALLS):
                t = stream.tile([128, Xc, CPK], f8, name="t",
                                tag=f"t{p}_{ci}", bufs=1)
                eng = nc.scalar if (dual_ring and qi % 2 == 1) else nc.sync
                qi += 1
                eng.dma_start(
                    t[:], xt[:, (p * N_K + c0) * CPK:(p * N_K + c0 + Xc) * CPK])
                if phase != "dma":
                    for k in range(Xc):
                        first = (ci == 0 and k == 0)
                        last = (ci == len(CALLS) - 1 and k == Xc - 1)
                        k_matmul(p, t, k, first, last)
                c0 += Xc
            if phase == "full":
                pair_epilogue(p)

        if phase in ("noop", "dma", "stream"):
            lout0 = ep.tile([1, 1], f32)
            nc.vector.memset(lout0[:], 0.0)
            nc.sync.dma_start(loss_d[:], lout0[:])
            bench_cm.__exit__(None, None, None)
            return

        # --- combine partials across the 8 cores ---
        # AllGather (floor ~4.6us on 8 cores) + local sum beats AllReduce
        # (floor ~9.7us) at this size.
        if with_collective:
            bf16 = mybir.dt.bfloat16
            # bf16 payload halves the bounce/AG/gather bytes; crl values are
            # O(1..8) sums of 4 ratios, bf16's ~0.4% per-entry error adds
            # ~3e-4 to the final loss rel-err (gate is 2e-2).
            crl16 = ep.tile([N, N], bf16)
            nc.vector.tensor_copy(crl16[:], crl[:])
            cc_in = dram.tile([N, N], bf16)
            cc_ag = dram.tile([N_CORES * N, N], bf16, addr_space="Shared")
            nc.sync.dma_start(cc_in[:], crl16[:])
            nc.gpsimd.collective_compute(
                "AllGather", Alu.bypass,
                replica_groups=[list(range(N_CORES))],
                ins=[cc_in.opt()], outs=[cc_ag.opt()],
            )
            # gather back as (64, r, 64): S[i, r, j] = AG[r*64+i, j]
            sg = ep.tile([N, N_CORES * N], bf16)
            nc.sync.dma_start(
                sg[:].rearrange("i (r j) -> i r j", r=N_CORES),
                cc_ag[:].rearrange("(r i) j -> i r j", r=N_CORES))
            crs = ep.tile([N, N], f32)
            # reduce over r: view free dim as (j outer, r inner) and reduce X
            nc.vector.tensor_reduce(
                crs[:], sg[:].rearrange("i (r j) -> i j r", r=N_CORES),
                mybir.AxisListType.X, Alu.add)
        else:
            crs = crl

        # --- final reduction ---
        u = ep.tile([N, N], f32)
        # u = (crs * 1/64) - beta
        nc.vector.scalar_tensor_tensor(u[:], crs[:], 1.0 / 64.0, beta_t[:],
                                       Alu.mult, Alu.subtract)
        v = ep.tile([N, N], f32)
        nc.vector.tensor_mul(v[:], u[:], wgt2_t[:])
        vr = ep.tile([N, 1], f32)
        nc.vector.tensor_reduce(vr[:], v[:], mybir.AxisListType.X, Alu.add,
                                apply_absolute_value=True)
        lps = tpsum.tile([1, 1], f32, padded_shape=[1, 512])
        nc.tensor.matmul(lps[:], lhsT=vr[:], rhs=ones_f32[:], start=True, stop=True)
        lout = ep.tile([1, 1], f32)
        nc.vector.tensor_copy(lout[:], lps[:])
        nc.sync.dma_start(loss_d[:], lout[:])

        bench_cm.__exit__(None, None, None)

    with tile.TileContext(nc) as tc:
        with tc.tile_pool(name="const", bufs=1) as const, \
             tc.tile_pool(name="stream", bufs=1) as stream, \
             tc.tile_pool(name="ep", bufs=1) as ep, \
             tc.tile_pool(name="gpsum", bufs=1, space="PSUM") as gpsum, \
             tc.tile_pool(name="tpsum", bufs=1, space="PSUM") as tpsum, \
             tc.tile_pool(name="dram", bufs=1, space="DRAM") as dram:
            emit(tc, const, stream, ep, gpsum, tpsum, dram)

    nc.compile()
    return nc


def _host_prep(masks: np.ndarray, nodes: np.ndarray):
    """Returns per-core input dicts (xt/beta/wgt2) for the 8 cores."""
    y = np.maximum(masks.astype(np.float32), -1.0) + 1.0      # (32,64,128,128) = 2m
    y8 = y.reshape(B, N, HW).astype(ml_dtypes.float8_e3m4)

    t = np.where(nodes < N_SEPARATE, 0, np.where(nodes < N_SEPARATE + N_FLEXIBLE, 1, 2))
    ti, tj = t[:, None], t[None, :]
    has_f = (ti == 1) | (tj == 1)
    has_a = (ti == 2) | (tj == 2)
    include = ~(has_f & ~has_a)
    beta = ((ti == 2) ^ (tj == 2)).astype(np.float32)
    triu = np.triu(np.ones((N, N), bool), k=1)
    wgt = (include & triu).astype(np.float64)
    wgt2 = ((wgt + wgt.T) / (2.0 * wgt.sum())).astype(np.float32)

    in_maps = []
    for c in range(N_CORES):
        # [q, h, node, k, p] -> [p, q, k, h, node]
        yv = y8[c * B_LOC:(c + 1) * B_LOC].reshape(N_PAIRS, 2, N, N_K, 128)
        yt = yv.transpose(4, 0, 3, 1, 2)       # [128, 2, 128, 2, 64]
        xt = np.empty((128, N_PAIRS, N_K, CPK), dtype=ml_dtypes.float8_e3m4)
        xt[..., 0:128] = yt.reshape(128, N_PAIRS, N_K, 128)
        xt[..., 128] = np.float32(1.0)
        in_maps.append({
            "xt": np.ascontiguousarray(xt.reshape(128, N_PAIRS * N_K * CPK)),
            "beta": beta, "wgt2": wgt2,
        })
    return in_maps


def kernel(masks: np.ndarray, nodes: np.ndarray) -> np.ndarray:
    from concourse.bass_utils import run_bass_kernel_spmd

    masks = np.asarray(masks, dtype=np.float32)
    nodes = np.asarray(nodes)
    in_maps = _host_prep(masks, nodes)

    if "nc" not in _cached:
        _cached["nc"] = _build_bass()
    nc = _cached["nc"]

    try:
        res = run_bass_kernel_spmd(nc, in_maps, core_ids=list(range(N_CORES)))
    except Exception:
        res = run_bass_kernel_spmd(nc, in_maps, core_ids=list(range(N_CORES)))
    loss = np.float32(res.results[0]["loss"][0, 0])
    return np.asarray(loss, dtype=np.float32).reshape(())


# revision 17
# speedup vs baseline: 1.0074x; 1.0074x over previous
"""Trainium2 Bass kernel for nn_FIoUCriterion (pairwise-overlap IoU-style loss).

Strategy (8 NeuronCores, data-parallel over batch), variant "fwl":
  - Host: y = max(x,-1)+1 (= 2*m), cast fp8 e3m4 (rel-err on final loss ~8e-5),
    pre-transpose per core to pixel-major layout [128, pair(2), ktile(128), 129]
    where each ktile block is [b_even nodes (64) | b_odd nodes (64) | ones].
  - Device (per core): per 128-pixel ktile and per 2-batch pair, ONE matmul
    with M=128 stacked weights (both batches) and N=129 streamed columns:
      out[128, 129] += block[:, 0:128].T @ block[:, 0:129]
    The 128-column weight enables Fast Weight Load (4 fp8/cycle), so the
    per-matmul cost is stream-bound (~N cycles) instead of load-bound --
    the old two-M=64-matmul structure paid two non-FWL 64-col loads per
    ktile (load-bound, ~2x slower).  Diagonal 64x64 blocks of the PSUM are
    the per-batch grams, col 128 is the mass sums s for both batches;
    off-diagonal blocks are unused cross-batch values (computed in the
    array's parallel rows -- zero extra cycles).
  - Epilogue per pair: C = gram/s_row per half, PE transposes per 64-block,
    cr_contrib = max(C, C^T) (valid since gram>=0), fold lower block via
    PE transpose (symmetric), AllGather + local-sum of the (64,64) partials
    across 8 cores, then loss = sum(|beta - cr_sum/64| * wgt2) with
    wgt2 = (wgt + wgt^T) / (2*sum(wgt)).
  - Scale bookkeeping: y = 2m  => gram_psum = 4*gram, s_psum = 2*s,
    C = 2*gram/s; sum over 32 batches then *1/64 gives mean cr.
  - DMA: plain contiguous loads, pair-major, alternated across the two HWDGE
    queues (SP + Activation).
"""

import numpy as np
import ml_dtypes

N_CORES = 8
B, N, H, W = 32, 64, 128, 128
HW = H * W
B_LOC = B // N_CORES          # 4 batches per core
N_PAIRS = B_LOC // 2          # 2 stacked pairs per core
N_K = HW // 128               # 128 ktiles of 128 pixels
CPK = 129                     # cols per ktile block (64 + 64 + ones)
N_SEPARATE = 7
N_FLEXIBLE = 2

_cached = {}


def _build_bass(with_collective: bool = True, bench_loop: int | None = None,
                phase: str = "full",
                calls_by_pair=None, dual_ring: bool = False,
                pe_order: str = "alt", c1_scalar: bool = True):
    import contextlib
    import concourse.bacc as bacc
    import concourse.mybir as mybir
    import concourse.tile as tile

    f32 = mybir.dt.float32
    f8 = mybir.dt.float8e3
    Alu = mybir.AluOpType

    nc = bacc.Bacc("TRN2", target_bir_lowering=False, debug=False, num_devices=N_CORES)
    xt = nc.dram_tensor("xt", [128, N_PAIRS * N_K * CPK], f8, kind="ExternalInput")
    beta_d = nc.dram_tensor("beta", [N, N], f32, kind="ExternalInput")
    wgt2_d = nc.dram_tensor("wgt2", [N, N], f32, kind="ExternalInput")
    loss_d = nc.dram_tensor("loss", [1, 1], f32, kind="ExternalOutput")

    def emit(tc, const, stream, ep, gpsum, tpsum, dram):
        # --- constants ---
        ones_f32 = const.tile([N, 1], f32)
        nc.vector.memset(ones_f32[:], 1.0)
        ident = const.tile([128, 128], f32)
        from concourse import masks as masks_lib
        masks_lib.make_identity(nc, ident[:])
        beta_t = const.tile([N, N], f32)
        nc.sync.dma_start(beta_t[:], beta_d[:])
        wgt2_t = const.tile([N, N], f32)
        nc.sync.dma_start(wgt2_t[:], wgt2_d[:])

        # --- PSUM accumulators per pair: [128, 129] = [gram blocks | s] ---
        g_acc = [gpsum.tile([128, CPK], f32, name=f"g_acc{p}",
                            padded_shape=[128, 512]) for p in range(N_PAIRS)]

        bench_cm = (tc.For_i(0, bench_loop, 1, hint_engines=(mybir.EngineType.PE,))
                    if bench_loop else contextlib.nullcontext())
        bench_cm.__enter__()

        f0 = ep.tile([N, N], f32)        # pair0's folded (64,64) partial
        a1 = ep.tile([N, N], f32)
        crl = ep.tile([N, N], f32)

        def k_matmul(p, t, k, first, last):
            block = t[:, k, :]           # [128, 129]
            nc.tensor.matmul(g_acc[p][:, 0:CPK],
                             lhsT=block[:, 0:128], rhs=block,
                             start=first, stop=last)

        def pair_epilogue(p):
            g = g_acc[p]
            # C[i,:] = gram_block(i)/s_i, stacked [128, 64]
            r = ep.tile([128, 1], f32, name=f"r{p}")
            nc.vector.reciprocal(r[:], g[:, 128:129])
            C = ep.tile([128, 64], f32, name=f"C{p}")
            nc.vector.tensor_scalar_mul(C[0:64, :], g[0:64, 0:64], r[0:64])
            nc.vector.tensor_scalar_mul(C[64:128, :], g[64:128, 64:128], r[64:128])
            # one full-width transpose: CT2[:, 0:64] = C_top^T, [:, 64:128] =
            # C_bot^T -- all on partitions 0:64 (transpose out must be at
            # PSUM partition 0)
            CT2 = tpsum.tile([64, 128], f32, name=f"CT{p}", tag="CT", bufs=1,
                             padded_shape=[64, 512])
            nc.tensor.transpose(CT2[:], C[:], ident[:])
            mxt = ep.tile([N, N], f32, name=f"mxt{p}")
            nc.vector.tensor_max(mxt[:], C[0:64, :], CT2[:, 0:64])
            # bottom block: copy C_bot^T to SBUF, re-transpose for the other
            # orientation, then max -- already folded to partitions 0:64
            # (copy on Activation so it overlaps the DVE max above)
            cpB = ep.tile([N, N], f32, name=f"cpB{p}")
            nc.scalar.copy(cpB[:], CT2[:, 64:128])
            TB = tpsum.tile([N, N], f32, name=f"TB{p}", tag="TB", bufs=1,
                            padded_shape=[64, 512])
            nc.tensor.transpose(TB[:], cpB[:], ident[0:64, 0:64])
            mxb = ep.tile([N, N], f32, name=f"mxb{p}")
            nc.vector.tensor_max(mxb[:], cpB[:], TB[:])
            if p == 0:
                # runs while pair1 is still streaming -- free
                nc.vector.tensor_add(f0[:], mxt[:], mxb[:])
            else:
                nc.vector.tensor_add(a1[:], f0[:], mxt[:])
                nc.vector.tensor_add(crl[:], a1[:], mxb[:])

        if phase == "pe":
            # pure PE-rate probe: one resident tile, full matmul count
            Xc = 16
            t = stream.tile([128, Xc, CPK], f8, name="tpe", tag="tpe", bufs=1)
            nc.sync.dma_start(t[:], xt[:, 0:Xc * CPK])
            if pe_order == "alt":
                for rep in range(N_K // Xc):
                    for k in range(Xc):
                        first = (rep == 0 and k == 0)
                        last = (rep == N_K // Xc - 1 and k == Xc - 1)
                        for p in range(N_PAIRS):
                            k_matmul(p, t, k, first, last)
            else:  # pair-major: replicate the stream phase's matmul order
                for p in range(N_PAIRS):
                    for rep in range(N_K // Xc):
                        for k in range(Xc):
                            first = (rep == 0 and k == 0)
                            last = (rep == N_K // Xc - 1 and k == Xc - 1)
                            k_matmul(p, t, k, first, last)
            lout0 = ep.tile([1, 1], f32)
            nc.vector.memset(lout0[:], 0.0)
            nc.sync.dma_start(loss_d[:], lout0[:])
            bench_cm.__exit__(None, None, None)
            return

        # --- streaming: plain contiguous DMA, pair-major; pair0's epilogue
        # overlaps pair1's matmul stream.  Calls alternate between the two
        # HWDGE queues (SP / Activation) when dual_ring. ---
        # Call shaping: ramped sizes; very large calls regress DMA throughput
        # (measured: a single 94-ktile call runs ~25% slower than 32-40 ktile
        # calls), so stay in the 12-40 ktile range after the pipe-fill call.
        CALLS_BY_PAIR = calls_by_pair or [
            [4, 12, 32, 40, 40],
            [40, 40, 32, 16],
        ]
        qi = 0
        for p in range(N_PAIRS if phase != "noop" else 0):
            CALLS = CALLS_BY_PAIR[p]
            assert sum(CALLS) == N_K
            c0 = 0
            for ci, Xc in enumerate(CALLS):
                t = stream.tile([128, Xc, CPK], f8, name="t",
                                tag=f"t{p}_{ci}", bufs=1)
                # Second call rides the otherwise-idle Activation HWDGE queue
                # so it transfers concurrently with call 0 (cuts the early
                # PE stall at the first call boundary).  Full alternation
                # regresses DMA throughput, so everything else stays on SP.
                if dual_ring:
                    eng = nc.scalar if qi % 2 == 1 else nc.sync
                elif c1_scalar:
                    eng = nc.scalar if (p == 0 and ci == 1) else nc.sync
                else:
                    eng = nc.sync
                qi += 1
                eng.dma_start(
                    t[:], xt[:, (p * N_K + c0) * CPK:(p * N_K + c0 + Xc) * CPK])
                if phase != "dma":
                    for k in range(Xc):
                        first = (ci == 0 and k == 0)
                        last = (ci == len(CALLS) - 1 and k == Xc - 1)
                        k_matmul(p, t, k, first, last)
                c0 += Xc
            if phase == "full":
                pair_epilogue(p)

        if phase in ("noop", "dma", "stream"):
            lout0 = ep.tile([1, 1], f32)
            nc.vector.memset(lout0[:], 0.0)
            nc.sync.dma_start(loss_d[:], lout0[:])
            bench_cm.__exit__(None, None, None)
            return

        # --- combine partials across the 8 cores ---
        # AllGather (floor ~4.6us on 8 cores) + local sum beats AllReduce
        # (floor ~9.7us) at this size.
        if with_collective:
            bf16 = mybir.dt.bfloat16
            # bf16 payload halves the bounce/AG/gather bytes; crl values are
            # O(1..8) sums of 4 ratios, bf16's ~0.4% per-entry error adds
            # ~3e-4 to the final loss rel-err (gate is 2e-2).
            crl16 = ep.tile([N, N], bf16)
            nc.vector.tensor_copy(crl16[:], crl[:])
            cc_in = dram.tile([N, N], bf16)
            cc_ag = dram.tile([N_CORES * N, N], bf16, addr_space="Shared")
            nc.sync.dma_start(cc_in[:], crl16[:])
            nc.gpsimd.collective_compute(
                "AllGather", Alu.bypass,
                replica_groups=[list(range(N_CORES))],
                ins=[cc_in.opt()], outs=[cc_ag.opt()],
            )
            # gather back as (64, r, 64): S[i, r, j] = AG[r*64+i, j]
            sg = ep.tile([N, N_CORES * N], bf16)
            nc.sync.dma_start(
                sg[:].rearrange("i (r j) -> i r j", r=N_CORES),
                cc_ag[:].rearrange("(r i) j -> i r j", r=N_CORES))
            crs = ep.tile([N, N], f32)
            # reduce over r: view free dim as (j outer, r inner) and reduce X
            nc.vector.tensor_reduce(
                crs[:], sg[:].rearrange("i (r j) -> i j r", r=N_CORES),
                mybir.AxisListType.X, Alu.add)
        else:
            crs = crl

        # --- final reduction ---
        u = ep.tile([N, N], f32)
        # u = (crs * 1/64) - beta
        nc.vector.scalar_tensor_tensor(u[:], crs[:], 1.0 / 64.0, beta_t[:],
                                       Alu.mult, Alu.subtract)
        v = ep.tile([N, N], f32)
        nc.vector.tensor_mul(v[:], u[:], wgt2_t[:])
        vr = ep.tile([N, 1], f32)
        nc.vector.tensor_reduce(vr[:], v[:], mybir.AxisListType.X, Alu.add,
                                apply_absolute_value=True)
        lps = tpsum.tile([1, 1], f32, padded_shape=[1, 512])
        nc.tensor.matmul(lps[:], lhsT=vr[:], rhs=ones_f32[:], start=True, stop=True)
        lout = ep.tile([1, 1], f32)
        nc.vector.tensor_copy(lout[:], lps[:])
        nc.sync.dma_start(loss_d[:], lout[:])

        bench_cm.__exit__(None, None, None)

    with tile.TileContext(nc) as tc:
        with tc.tile_pool(name="const", bufs=1) as const, \
             tc.tile_pool(name="stream", bufs=1) as stream, \
             tc.tile_pool(name="ep", bufs=1) as ep, \
             tc.tile_pool(name="gpsum", bufs=1, space="PSUM") as gpsum, \
             tc.tile_pool(name="tpsum", bufs=1, space="PSUM") as tpsum, \
             tc.tile_pool(name="dram", bufs=1, space="DRAM") as dram:
            emit(tc, const, stream, ep, gpsum, tpsum, dram)

    nc.compile()
    return nc


def _host_prep(masks: np.ndarray, nodes: np.ndarray):
    """Returns per-core input dicts (xt/beta/wgt2) for the 8 cores."""
    y = np.maximum(masks.astype(np.float32), -1.0) + 1.0      # (32,64,128,128) = 2m
    y8 = y.reshape(B, N, HW).astype(ml_dtypes.float8_e3m4)

    t = np.where(nodes < N_SEPARATE, 0, np.where(nodes < N_SEPARATE + N_FLEXIBLE, 1, 2))
    ti, tj = t[:, None], t[None, :]
    has_f = (ti == 1) | (tj == 1)
    has_a = (ti == 2) | (tj == 2)
    include = ~(has_f & ~has_a)
    beta = ((ti == 2) ^ (tj == 2)).astype(np.float32)
    triu = np.triu(np.ones((N, N), bool), k=1)
    wgt = (include & triu).astype(np.float64)
    wgt2 = ((wgt + wgt.T) / (2.0 * wgt.sum())).astype(np.float32)

    in_maps = []
    for c in range(N_CORES):
        # [q, h, node, k, p] -> [p, q, k, h, node]
        yv = y8[c * B_LOC:(c + 1) * B_LOC].reshape(N_PAIRS, 2, N, N_K, 128)
        yt = yv.transpose(4, 0, 3, 1, 2)       # [128, 2, 128, 2, 64]
        xt = np.empty((128, N_PAIRS, N_K, CPK), dtype=ml_dtypes.float8_e3m4)
        xt[..., 0:128] = yt.reshape(128, N_PAIRS, N_K, 128)
        xt[..., 128] = np.float32(1.0)
        in_maps.append({
            "xt": np.ascontiguousarray(xt.reshape(128, N_PAIRS * N_K * CPK)),
            "beta": beta, "wgt2": wgt2,
        })
    return in_maps


def kernel(masks: np.ndarray, nodes: np.ndarray) -> np.ndarray:
    from concourse.bass_utils import run_bass_kernel_spmd

    masks = np.asarray(masks, dtype=np.float32)
    nodes = np.asarray(nodes)
    in_maps = _host_prep(masks, nodes)

    if "nc" not in _cached:
        _cached["nc"] = _build_bass()
    nc = _cached["nc"]

    try:
        res = run_bass_kernel_spmd(nc, in_maps, core_ids=list(range(N_CORES)))
    except Exception:
        res = run_bass_kernel_spmd(nc, in_maps, core_ids=list(range(N_CORES)))
    loss = np.float32(res.results[0]["loss"][0, 0])
    return np.asarray(loss, dtype=np.float32).reshape(())
